# revision 8
# baseline (speedup 1.0000x reference)
"""Causal multi-head attention block (B=1, S=4096, D=1024, H=16, dh=64) on 8 TRN2 cores.

Sharding: tensor-parallel over heads -- 2 heads per core. Each core computes its
2 heads' contribution to the output projection; the host sums the 8 partials
(the "all-reduce" is a free host-side add) and adds b_proj.

Per-core kernel (all matmuls bf16, fp32 PSUM accumulation):
  1. qkvT = W_shard^T @ x  computed as [384, 4096] with W tiles stationary and
     host-pretransposed xT streaming. q columns pre-scaled by 1/8 on host.
  2. vT tiles are PE-transposed into v_nat and augmented with a ones column
     ([keys, 64+1]) so the PV matmul also produces softmax row sums.
  3. Flash-style causal attention in scoresT ([keys, q]) layout:
     QK^T row-tiled (two dh=64 heads concurrent), exp on ScalarE with a fixed
     -15 bias shift (cancels in normalization), PV col-tiled as M=32/32/33+33.
  4. Row sums land in PSUM; reciprocal + partition-broadcast multiply
     normalizes aT during PSUM->SBUF assembly.
  5. out_partial = a^T.T @ W_proj_shard, written as bf16.
"""

import os
import sys

import numpy as np

sys.path.insert(0, "/opt/trn_rl_repo")

B, S, D, H = 1, 4096, 1024, 16
DH = 64
NCORES = 8

_COMPILED = {}


def _build_bass():
    import concourse.bass as bass
    import concourse.tile as tile
    from concourse import bacc, mybir

    f32 = mybir.dt.float32
    bf16 = mybir.dt.bfloat16

    nc = bacc.Bacc("TRN2", target_bir_lowering=False, debug=False)

    xT_d = nc.dram_tensor("xT", [D, S], bf16, kind="ExternalInput").ap()
    wqkv_d = nc.dram_tensor("wqkv", [D, 384], bf16, kind="ExternalInput").ap()
    bqkv_d = nc.dram_tensor("bqkv", [128, 3], f32, kind="ExternalInput").ap()
    wp_d = nc.dram_tensor("wp", [128, D], bf16, kind="ExternalInput").ap()
    eye_d = nc.dram_tensor("eye", [128, 128], bf16, kind="ExternalInput").ap()
    wedge_d = nc.dram_tensor("wedge", [128, 128], f32, kind="ExternalInput").ap()
    out_d = nc.dram_tensor("out", [S, D], bf16, kind="ExternalOutput").ap()

    NKT = S // 128  # 32 key tiles
    NQC = S // 512  # 8 query chunks
    Exp = mybir.ActivationFunctionType.Exp

    with tile.TileContext(nc) as tc:
        from contextlib import ExitStack

        with ExitStack() as ctx:
            const = ctx.enter_context(tc.tile_pool(name="const", bufs=1))
            eye_sb = const.tile([128, 128], bf16)
            nc.sync.dma_start(eye_sb[:], eye_d[:])
            wedge_sb = const.tile([128, 128], f32)
            nc.sync.dma_start(wedge_sb[:], wedge_d[:])
            w_sb = const.tile([128, 8 * 384], bf16)
            for k in range(8):
                nc.sync.dma_start(
                    w_sb[:, k * 384 : (k + 1) * 384], wqkv_d[k * 128 : (k + 1) * 128, :]
                )
            bias_sb = const.tile([128, 3], f32)
            nc.sync.dma_start(bias_sb[:], bqkv_d[:])
            zbias = const.tile([128, 1], f32)
            nc.vector.memset(zbias[:], 0.0)
            ones_row = const.tile([1, 64], f32)
            nc.vector.memset(ones_row[:], 1.0)
            wp_sb = const.tile([128, D], bf16)
            nc.sync.dma_start(wp_sb[:], wp_d[:])

            # xT chunks, n-major so the first (m,n) k-accumulation starts early
            xT_sb = const.tile([128, 8 * S], bf16)  # [k][n*512]
            for n in range(8):
                for k in range(8):
                    nc.sync.dma_start(
                        xT_sb[:, k * S + n * 512 : k * S + (n + 1) * 512],
                        xT_d[k * 128 : (k + 1) * 128, n * 512 : (n + 1) * 512],
                    )

            qkv_sb = const.tile([128, 3 * S], bf16)  # [m][seq]: m0=qT m1=kT m2=vT
            vaugA = const.tile([128, NKT * 65], bf16)
            vaugB = const.tile([128, NKT * 65], bf16)

            # ---- Phase 1: qkvT [384, 4096] ----
            with tc.tile_pool(name="qkv_ps", bufs=3, space="PSUM") as qkv_ps:
                for m in range(3):
                    for n in range(8):
                        ps = qkv_ps.tile([128, 512], f32)
                        for k in range(8):
                            nc.tensor.matmul(
                                ps[:],
                                lhsT=w_sb[:, k * 384 + m * 128 : k * 384 + (m + 1) * 128],
                                rhs=xT_sb[:, k * S + n * 512 : k * S + (n + 1) * 512],
                                start=(k == 0),
                                stop=(k == 7),
                            )
                        nc.vector.tensor_scalar_add(
                            qkv_sb[:, m * S + n * 512 : m * S + (n + 1) * 512],
                            ps[:],
                            bias_sb[:, m : m + 1],
                        )

            # ---- Phase 2: v_nat augmented tiles ----
            with tc.tile_pool(name="vt_ps", bufs=3, space="PSUM") as vt_ps:
                for kt in range(NKT):
                    pst = vt_ps.tile([128, 128], bf16)
                    nc.tensor.transpose(
                        pst[:], qkv_sb[:, 2 * S + kt * 128 : 2 * S + (kt + 1) * 128], eye_sb[:]
                    )
                    nc.vector.tensor_copy(vaugA[:, kt * 65 : kt * 65 + 64], pst[:, 0:64])
                    nc.vector.tensor_copy(vaugB[:, kt * 65 : kt * 65 + 64], pst[:, 64:128])
                    nc.vector.memset(vaugA[:, kt * 65 + 64 : kt * 65 + 65], 1.0)
                    nc.vector.memset(vaugB[:, kt * 65 + 64 : kt * 65 + 65], 1.0)

            # ---- Phase 3: causal flash attention + projection ----
            sc_pool = ctx.enter_context(tc.tile_pool(name="sc_ps", bufs=2, space="PSUM"))
            at_pool = ctx.enter_context(tc.tile_pool(name="at_ps", bufs=1, space="PSUM"))
            pj_pool = ctx.enter_context(tc.tile_pool(name="pj_ps", bufs=2, space="PSUM"))
            p_pool = ctx.enter_context(tc.tile_pool(name="p_sb", bufs=4))
            asb_pool = ctx.enter_context(tc.tile_pool(name="asb", bufs=2))
            rc_pool = ctx.enter_context(tc.tile_pool(name="rc", bufs=4))
            ot_pool = ctx.enter_context(tc.tile_pool(name="ot", bufs=4))

            def qT(lo_p, hi_p, c0, c1):
                return qkv_sb[lo_p:hi_p, c0:c1]

            def kT(lo_p, hi_p, c0, c1):
                return qkv_sb[lo_p:hi_p, S + c0 : S + c1]

            for qc in range(NQC):
                nkt = (qc + 1) * 4
                aT = at_pool.tile([128, 1024], f32)  # [:,0:512]=bank1 [:,512:]=bank2
                for kt in range(nkt):
                    ktl = kt - qc * 4
                    lo = max(ktl, 0) * 128
                    q0 = qc * 512
                    sc = sc_pool.tile([128, 1024], f32)
                    # QK^T: both heads concurrent via row groups
                    nc.tensor.matmul(
                        sc[:, lo:512],
                        lhsT=kT(0, 64, kt * 128, (kt + 1) * 128),
                        rhs=qT(0, 64, q0 + lo, q0 + 512),
                        start=True,
                        stop=True,
                        tile_position=(0, 0),
                    )
                    nc.tensor.matmul(
                        sc[:, 512 + lo : 1024],
                        lhsT=kT(64, 128, kt * 128, (kt + 1) * 128),
                        rhs=qT(64, 128, q0 + lo, q0 + 512),
                        start=True,
                        stop=True,
                        tile_position=(64, 0),
                    )
                    if ktl >= 0:
                        nc.vector.tensor_add(
                            sc[:, lo : lo + 128], sc[:, lo : lo + 128], wedge_sb[:]
                        )
                        nc.vector.tensor_add(
                            sc[:, 512 + lo : 512 + lo + 128],
                            sc[:, 512 + lo : 512 + lo + 128],
                            wedge_sb[:],
                        )
                    P = p_pool.tile([128, 1024], bf16)
                    if ktl >= 0:
                        nc.scalar.activation(P[:, lo:512], sc[:, lo:512], Exp, bias=zbias[:])
                        nc.scalar.activation(
                            P[:, 512 + lo : 1024], sc[:, 512 + lo : 1024], Exp, bias=zbias[:]
                        )
                    else:
                        nc.scalar.activation(P[:], sc[:], Exp, bias=zbias[:])
                    st, sp = (kt == 0), (kt == nkt - 1)
                    ko = kt * 65
                    # PV: slot1 = three concurrent col-tiled matmuls; slot2 = P4
                    nc.tensor.matmul(
                        aT[0:32, lo:512],
                        lhsT=vaugA[:, ko : ko + 32],
                        rhs=P[:, lo:512],
                        start=st,
                        stop=sp,
                        tile_position=(0, 0),
                    )
                    nc.tensor.matmul(
                        aT[32:64, lo:512],
                        lhsT=vaugB[:, ko : ko + 32],
                        rhs=P[:, 512 + lo : 1024],
                        start=st,
                        stop=sp,
                        tile_position=(0, 32),
                        skip_group_check=True,
                    )
                    nc.tensor.matmul(
                        aT[64:97, lo:512],
                        lhsT=vaugA[:, ko + 32 : ko + 65],
                        rhs=P[:, lo:512],
                        start=st,
                        stop=sp,
                        tile_position=(0, 64),
                        skip_group_check=True,
                    )
                    nc.tensor.matmul(
                        aT[0:33, 512 + lo : 1024],
                        lhsT=vaugB[:, ko + 32 : ko + 65],
                        rhs=P[:, 512 + lo : 1024],
                        start=st,
                        stop=sp,
                        tile_position=(0, 0),
                    )
                # normalize + assemble aT_sb [hd, 512]
                recA = rc_pool.tile([1, 512], f32)
                recB = rc_pool.tile([1, 512], f32)
                nc.vector.reciprocal(recA[:], aT[96:97, 0:512])
                nc.vector.reciprocal(recB[:], aT[32:33, 512:1024])
                # broadcast recips to 32 partitions via a K=1 ones matmul
                bc_ps = sc_pool.tile([64, 512], f32, tag="sc")
                nc.tensor.matmul(
                    bc_ps[0:32, :], lhsT=ones_row[0:1, 0:32], rhs=recA[:],
                    start=True, stop=True, tile_position=(0, 0),
                )
                nc.tensor.matmul(
                    bc_ps[32:64, :], lhsT=ones_row[0:1, 0:32], rhs=recB[:],
                    start=True, stop=True, tile_position=(0, 32),
                    skip_group_check=True,
                )
                bc_sb = rc_pool.tile([64, 512], f32)
                nc.vector.tensor_copy(bc_sb[:], bc_ps[:])
                aT_sb = asb_pool.tile([128, 512], bf16)
                nc.vector.tensor_mul(aT_sb[0:32, :], aT[0:32, 0:512], bc_sb[0:32, :])
                nc.vector.tensor_mul(aT_sb[32:64, :], aT[64:96, 0:512], bc_sb[0:32, :])
                nc.vector.tensor_mul(aT_sb[64:96, :], aT[32:64, 0:512], bc_sb[32:64, :])
                nc.vector.tensor_mul(aT_sb[96:128, :], aT[0:32, 512:1024], bc_sb[32:64, :])
                # projection for this chunk
                for j in range(4):
                    for h in range(2):
                        pps = pj_pool.tile([128, 512], f32)
                        nc.tensor.matmul(
                            pps[:],
                            lhsT=aT_sb[:, j * 128 : (j + 1) * 128],
                            rhs=wp_sb[:, h * 512 : (h + 1) * 512],
                            start=True,
                            stop=True,
                        )
                        ot = ot_pool.tile([128, 512], bf16)
                        nc.vector.tensor_copy(ot[:], pps[:])
                        nc.sync.dma_start(
                            out_d[
                                (qc * 4 + j) * 128 : (qc * 4 + j + 1) * 128,
                                h * 512 : (h + 1) * 512,
                            ],
                            ot[:],
                        )

    nc.compile()
    return nc


def _get_compiled():
    if "nc" not in _COMPILED:
        _COMPILED["nc"] = _build_bass()
    return _COMPILED["nc"]


def _host_inputs(x, mask, W_attn, b_attn, W_proj, b_proj):
    import ml_dtypes

    bf16 = ml_dtypes.bfloat16
    xT = np.ascontiguousarray(x.reshape(S, D).T).astype(bf16)  # [D, S]
    eye = np.eye(128, dtype=bf16)
    j = np.arange(128)
    wedge = np.where(j[None, :] >= j[:, None], 0.0, -1e5).astype(np.float32)
    in_maps = []
    for c in range(NCORES):
        hs = slice(2 * c * DH, 2 * c * DH + 128)
        wq = W_attn[:, 0:D][:, hs] * 0.125
        wk = W_attn[:, D : 2 * D][:, hs]
        wv = W_attn[:, 2 * D : 3 * D][:, hs]
        wqkv = np.concatenate([wq, wk, wv], axis=1).astype(bf16)
        bq = b_attn[0:D][hs] * 0.125
        bk = b_attn[D : 2 * D][hs]
        bv = b_attn[2 * D : 3 * D][hs]
        bqkv = np.stack([bq, bk, bv], axis=1).astype(np.float32)  # [128, 3]
        wp = np.ascontiguousarray(W_proj[hs, :]).astype(bf16)
        in_maps.append(
            {
                "xT": xT,
                "wqkv": wqkv,
                "bqkv": bqkv,
                "wp": wp,
                "eye": eye,
                "wedge": wedge,
            }
        )
    return in_maps


def _run(in_maps, trace=False):
    from concourse.bass_utils import run_bass_kernel_spmd

    nc = _get_compiled()
    res = run_bass_kernel_spmd(nc, in_maps, list(range(NCORES)), trace=trace)
    return res


def kernel(x, mask, W_attn, b_attn, W_proj, b_proj):
    x = np.asarray(x, dtype=np.float32)
    W_attn = np.asarray(W_attn, dtype=np.float32)
    b_attn = np.asarray(b_attn, dtype=np.float32)
    W_proj = np.asarray(W_proj, dtype=np.float32)
    b_proj = np.asarray(b_proj, dtype=np.float32)
    in_maps = _host_inputs(x, mask, W_attn, b_attn, W_proj, b_proj)
    res = _run(in_maps)
    acc = np.zeros((S, D), dtype=np.float32)
    for c in range(NCORES):
        acc += res.results[c]["out"].astype(np.float32)
    acc += b_proj[None, :]
    return acc.reshape(B, S, D)
